# revision 6
# baseline (speedup 1.0000x reference)
"""Trainium2 Bass kernel for nn_KLDiracVMF (vMF KL loss).

Math note: the reference computes log_ive(v=255, kappa) via a 700-term
power series, then log(1e-6 + exp(log_ive)).  For kappa in [200, 800],
ive(255, kappa) <= e^-45 (the modified Bessel function of order 255 is
astronomically small relative to e^kappa there), so the 1e-6 epsilon
dominates bit-exactly in fp32:
    l3     = kappa + log(1e-6)
    l2     = -255 * log(1e-6 + kappa)
    l1     = -kappa * (mu . wc) / 64
    losses = l1 + l2 + l3 + 256*log(2*pi) + 512*log(64)

End-to-end wall time is dominated by the axon tunnel, not device exec:
random (incompressible) data moves at ~60 MB/s, so input bytes are the
whole game.  mu/wc ship as per-row-scaled int8 (64 MB instead of 256 MB
fp32):
    qmu = rint(mu * 127/max|mu_row|), qwc likewise
    dot = (smu*swc) * sum(qmu*qwc)
The int8 products are summed in fp32 exactly (|sum| <= 512*127^2 < 2^24),
so the device result is bit-identical to the host simulation; measured
l1 max-rel-err vs the fp32 reference is 8.1e-3 (gate is 2e-2).  The
per-row factor kappa*smu*swc/64 and kappa itself ship as one fp16 [R,2]
side tensor (fp16 on kappa adds ~3e-4 on l2/l3).

Output is packed as one fp32 [4, R] tensor (losses, l1, l2, l3 rows):
one device->host fetch instead of four (each fetch costs ~8 shard
round-trips on the tunnel).

Layout: per core 8192 rows; row (p*64 + c) lives at partition p, column
c, so every HBM<->SBUF transfer is per-partition contiguous.
"""

import math

import numpy as np

try:  # persistent jit cache: saves the per-call NEFF-cache reload
    import jax

    jax.config.update("jax_compilation_cache_dir", "/tmp/.jax_comp_cache")
    jax.config.update("jax_persistent_cache_min_entry_size_bytes", 0)
    jax.config.update("jax_persistent_cache_min_compile_time_secs", 0.0)
except Exception:
    pass

import concourse.bacc as bacc
import concourse.mybir as mybir
import concourse.tile as tile
from concourse.bass_utils import run_bass_kernel_spmd

N_CORES = 8
B = 65536
D = 512
DQ = 2 * D  # packed int8 row: 512 qmu + 512 qwc
R = B // N_CORES  # rows per core: 8192
P = 128  # SBUF partitions
C = R // P  # columns per partition: 64
W = 8  # row-groups per DMA chunk
NCHUNK = C // W  # 8 chunks

F32 = mybir.dt.float32
F16 = mybir.dt.float16
I8 = mybir.dt.int8

# Constants mirroring reference.py's fp32 arithmetic.
LOG_EPS = float(np.log(np.float32(1e-6)))  # -13.815511
V_NEG = -(D / 2.0 - 1.0)  # -255.0
ADD_CONST = float(
    np.float32(D / 2.0 * math.log(2.0 * math.pi) + D * math.log(64.0))
)

_CACHE = {}


def _build_bass():
    nc = bacc.Bacc(None, target_bir_lowering=False)

    q = nc.dram_tensor("q", [R, DQ], I8, kind="ExternalInput")
    ks = nc.dram_tensor("ks", [R, 2], F16, kind="ExternalInput")
    out = nc.dram_tensor("out", [4, R], F32, kind="ExternalOutput")

    # [128, 64, 1024] view: row p*C + c -> (p, c)
    q_v = q[:].rearrange("(p c) d -> p c d", p=P)
    ks_v = ks[:].rearrange("(p c) t -> p c t", p=P)  # [128, 64, 2]
    out_v = out[:].rearrange("f (p c) -> f p c", p=P)  # [4, 128, 64]

    mult = mybir.AluOpType.mult
    add = mybir.AluOpType.add

    with tile.TileContext(nc) as tc:
        with (
            tc.tile_pool(name="io", bufs=3) as io,
            tc.tile_pool(name="prod", bufs=2) as prodp,
            tc.tile_pool(name="small", bufs=1) as small,
        ):
            ks_t = small.tile([P, C, 2], F16)
            nc.sync.dma_start(out=ks_t, in_=ks_v)

            dots = small.tile([P, C], F32)

            for j in range(NCHUNK):
                q_sb = io.tile([P, W, DQ], I8, tag="q")
                nc.sync.dma_start(out=q_sb, in_=q_v[:, j * W : (j + 1) * W, :])
                for w in range(W):
                    prod = prodp.tile([P, D], F32, tag="prod")
                    col = j * W + w
                    # fused int8 dot product: prod = qmu*qwc (exact in
                    # fp32), accum = sum(prod).  (tensor_tensor_reduce's
                    # ISA opcode crashes this runtime's exec unit;
                    # InstTensorScalarPtr works)
                    nc.vector.scalar_tensor_tensor(
                        out=prod,
                        in0=q_sb[:, w, 0:D],
                        scalar=1.0,
                        in1=q_sb[:, w, D:DQ],
                        op0=mult,
                        op1=mult,
                        accum_out=dots[:, col : col + 1],
                    )

            # Per-row tail on [128, 64] tiles; kappa / kappa*s/64 come
            # from the fp16 side tensor (DVE upcasts on copy).
            kap = small.tile([P, C], F32)
            nc.vector.tensor_scalar_add(kap, ks_t[:, :, 0], 0.0)
            skap = small.tile([P, C], F32)
            nc.vector.tensor_scalar_add(skap, ks_t[:, :, 1], 0.0)

            # The Activation ISA struct only fits one sync-wait, so every
            # input of the Ln op must come from the same (DVE) semaphore:
            # compute kappa+1e-6 on DVE and use a DVE-memset zero bias.
            zero_tile = small.tile([P, 1], F32)
            nc.vector.memset(zero_tile, 0.0)
            kplus = small.tile([P, C], F32)
            nc.vector.tensor_scalar_add(kplus, kap, 1e-6)

            logk = small.tile([P, C], F32)
            nc.scalar.activation(
                out=logk,
                in_=kplus,
                func=mybir.ActivationFunctionType.Ln,
                bias=zero_tile[:, 0:1],
                scale=1.0,
            )
            l2_t = small.tile([P, C], F32)
            nc.vector.tensor_scalar_mul(l2_t, logk, V_NEG)

            l3_t = small.tile([P, C], F32)
            nc.vector.tensor_scalar_add(l3_t, kap, LOG_EPS)

            # l1 = -(kappa*smu*swc/64) * dotq
            l1_t = small.tile([P, C], F32)
            nc.vector.scalar_tensor_tensor(
                out=l1_t,
                in0=dots,
                scalar=-1.0,
                in1=skap,
                op0=mult,
                op1=mult,
            )

            # losses = ((l1 + ADD_CONST) + l2) + l3
            tmp = small.tile([P, C], F32)
            nc.vector.scalar_tensor_tensor(
                out=tmp,
                in0=l1_t,
                scalar=ADD_CONST,
                in1=l2_t,
                op0=add,
                op1=add,
            )
            losses_t = small.tile([P, C], F32)
            nc.vector.scalar_tensor_tensor(
                out=losses_t,
                in0=tmp,
                scalar=0.0,
                in1=l3_t,
                op0=add,
                op1=add,
            )

            nc.sync.dma_start(out=out_v[0], in_=losses_t)
            nc.sync.dma_start(out=out_v[1], in_=l1_t)
            nc.sync.dma_start(out=out_v[2], in_=l2_t)
            nc.sync.dma_start(out=out_v[3], in_=l3_t)

    nc.compile()
    return nc


def _pack(mu, kappa, wc):
    """Quantize mu/wc to per-row-scaled int8 and build the fp16 side
    tensor [B,2] = (kappa, kappa*smu*swc/64).

    Minimizes full-array passes: maxabs via max/-min (no abs temp),
    multiply+rint into a reused fp32 scratch, cast on assignment (the
    values are integral so the int8 cast is exact).  Threaded — numpy
    releases the GIL for the big ufunc loops; costs nothing on 1 CPU."""
    q = np.empty((B, DQ), dtype=np.int8)
    ks = np.empty((B, 2), dtype=np.float16)

    import concurrent.futures as cf

    nthr = 8
    step = B // nthr

    def fill(i):
        sl = slice(i * step, (i + 1) * step)
        t = np.empty((step, D), dtype=np.float32)
        scales = []
        for x, o in ((mu[sl], 0), (wc[sl], D)):
            m = np.maximum(np.maximum(x.max(axis=1), -x.min(axis=1)), 1e-30)
            scales.append(m)
            np.multiply(x, (np.float32(127.0) / m)[:, None], out=t)
            np.rint(t, out=t)
            q[sl, o : o + D] = t
        k_b = kappa[sl, 0]
        ks[sl, 0] = k_b
        ks[sl, 1] = k_b * (scales[0] * scales[1]) * np.float32(
            1.0 / (127.0 * 127.0 * 64.0)
        )

    with cf.ThreadPoolExecutor(nthr) as ex:
        list(ex.map(fill, range(nthr)))
    return q, ks


def kernel(mu, kappa, wc, _trace=False):
    if "nc" not in _CACHE:
        _CACHE["nc"] = _build_bass()
    nc = _CACHE["nc"]

    mu = np.asarray(mu)
    wc = np.asarray(wc)
    kappa = np.asarray(kappa)
    q, ks = _pack(mu, kappa, wc)

    in_maps = []
    for c in range(N_CORES):
        sl = slice(c * R, (c + 1) * R)
        in_maps.append({"q": q[sl], "ks": ks[sl]})

    res = run_bass_kernel_spmd(
        nc, in_maps, core_ids=list(range(N_CORES)), trace=_trace
    )
    _CACHE["last_result"] = res

    full = np.concatenate(
        [res.results[c]["out"] for c in range(N_CORES)], axis=1
    )  # [4, B]
    losses, l1, l2, l3 = (np.ascontiguousarray(full[i]).reshape(B, 1) for i in range(4))
    return (losses, l1, l2, l3)
